# revision 14
# baseline (speedup 1.0000x reference)
"""Multi-head attention TRN2 kernel: 8-core head-sharded tensor parallelism.

Full inputs in, full output out. Each core computes 2 of the 16 heads:
QKV projection (its column slice), flash-style attention, and a partial
out-projection against its row slice of Wo. Host sums the 8 partials and
adds (bv @ Wo + bo) once; the K bias is dropped entirely (it only adds a
per-query constant to the logits, which softmax cancels).

v4 (all bf16, PE-bound): the PE carries ~1.45 us/step of work vs the
scalar engine's ~1.15 us exp, so the schedule's job is to keep the PE
queue dense with zero bursts. Background projections are half-unit
slices (~1 us) placed on a precomputed per-step plan (<=2 per step,
deadline-feasible, front-loaded); norm and out-projection drain at a
bounded per-step rate. Score pairs run concurrently on disjoint PE row
groups; ctx stationaries are zero-padded to 128 columns for FWL.
Prologue: dummy matmuls on memset data ramp the PE pstate while the
first x DMA lands. Tail: the last q-block's outproj copies ride the
then-idle scalar engine instead of the DVE.
"""
import sys

sys.path.insert(0, "/opt/trn_rl_repo")

from collections import deque
from contextlib import ExitStack

import numpy as np

import concourse.bass as bass
import concourse.tile as tile
from concourse import bacc, mybir
from concourse.bass_utils import run_bass_kernel_spmd

f32 = mybir.dt.float32
bf16 = mybir.dt.bfloat16
EXP = mybir.ActivationFunctionType.Exp

N_CORES = 8
B, S, F = 2, 2048, 1024
H = 16                 # heads total
DK = F // H            # 64
HPC = H // N_CORES     # 2 heads per core
CF = HPC * DK          # 128 = per-core slice of features
T = B * S              # 4096 tokens
TU = 512               # tokens per projection unit
NU = T // TU           # 8 projection units
NKT = S // 128         # 16 key tiles per sequence
NQB = S // 512         # 4 q-blocks per sequence
NC = F // 128          # 8 contraction chunks
CTX_LAG = 6


def build_program():
    nc = bacc.Bacc("TRN2", target_bir_lowering=False, debug=False,
                   num_devices=N_CORES)

    xt_d = nc.dram_tensor("xT", [F, T], bf16, kind="ExternalInput").ap()
    wq_d = nc.dram_tensor("Wq", [F, CF], bf16, kind="ExternalInput").ap()
    wk_d = nc.dram_tensor("Wk", [F, CF], bf16, kind="ExternalInput").ap()
    wv_d = nc.dram_tensor("Wv", [F, CF], bf16, kind="ExternalInput").ap()
    bq_d = nc.dram_tensor("bq", [CF, 1], f32, kind="ExternalInput").ap()
    wo_d = nc.dram_tensor("Wo", [CF, F], bf16, kind="ExternalInput").ap()
    yp_d = nc.dram_tensor("yp", [T, F], bf16, kind="ExternalOutput").ap()

    with tile.TileContext(nc) as tc, ExitStack() as ctx:
        const = ctx.enter_context(tc.tile_pool(name="const", bufs=1))
        big = ctx.enter_context(tc.tile_pool(name="big", bufs=1))
        etp = ctx.enter_context(tc.tile_pool(name="etp", bufs=9))
        csb = ctx.enter_context(tc.tile_pool(name="csb", bufs=2))
        small = ctx.enter_context(tc.tile_pool(name="small", bufs=4))
        ypool = ctx.enter_context(tc.tile_pool(name="ypool", bufs=4))

        # ---- persistent activations (all bf16) ----
        qt_sb = big.tile([128, T], bf16)       # [2 heads x 64 d, tokens]
        kt_sb = big.tile([128, T], bf16)
        # vaug per (b,kt,h): 128 cols = [V_h (64) | ones (1) | zeros (63)]
        # -> full-128-column stationary triggers FWL; ctx psum rows 65-127
        # are zeros and ignored.
        vaug_sb = big.tile([128, B, NKT, 2, 128], bf16)
        ctx2t_sb = big.tile([128, B, S], bf16)  # [2 heads x 64 d, b, tokens]

        warm_sb = const.tile([128, 512], bf16)
        nc.vector.memset(warm_sb, 0.0)
        ones_bf = const.tile([1, 64], bf16)
        nc.vector.memset(ones_bf, 1.0)

        # ---- inputs: DMA order puts the step-0 critical path first ----
        xall = const.tile([128, NU, NC, TU], bf16)
        wq_sb = const.tile([128, NC, CF], bf16)
        wk_sb = const.tile([128, NC, CF], bf16)
        wv_sb = const.tile([128, NC, CF], bf16)
        bq_sb = const.tile([128, 1], f32)
        wo_sb = const.tile([128, F], bf16)

        def dma_x(j):
            nc.sync.dma_start(
                xall[:, j, :, :],
                xt_d[:, j * TU:(j + 1) * TU]
                .rearrange("(a p) t -> p a t", p=128))

        dmadummy = const.tile([1, 4, 8], bf16)
        dma_x(0)
        nc.sync.dma_start(wq_sb, wq_d.rearrange("(a p) n -> p a n", p=128))
        nc.sync.dma_start(wk_sb, wk_d.rearrange("(a p) n -> p a n", p=128))
        nc.sync.dma_start(bq_sb, bq_d)
        # ring fillers: the DMA descriptor ring is ~8 deep, so these make
        # every later descriptor wait for x0's completion -> unit 0 gets
        # the full fabric bandwidth during the cold start
        for dd in range(4):
            nc.sync.dma_start(dmadummy[:, dd, :], xt_d[0:1, 0:8])
        dma_x(1)
        nc.sync.dma_start(wv_sb, wv_d.rearrange("(a p) n -> p a n", p=128))
        dma_x(2)
        nc.sync.dma_start(wo_sb, wo_d)
        for j in range(3, NU):
            dma_x(j)

        # ---- PSUM pools: 4 + 2 + 2 banks ----
        sc_ps = ctx.enter_context(
            tc.tile_pool(name="sc_ps", bufs=2, space="PSUM"))   # 4 banks
        pc_ps = ctx.enter_context(
            tc.tile_pool(name="pc_ps", bufs=2, space="PSUM"))   # 2 banks
        aux_ps = ctx.enter_context(
            tc.tile_pool(name="aux_ps", bufs=2, space="PSUM"))  # 2 banks

        # ---- PE warm-up: ramp the pstate while the x DMA lands ----
        pw = aux_ps.tile([128, 512], f32, tag="aux", name="warm")
        for w in range(13):
            nc.tensor.matmul(pw, warm_sb[:, 0:128], warm_sb,
                             start=(w == 0), stop=(w == 12))
        warm_sink = small.tile([1, 1], f32, tag="rs", name="warmsink")
        nc.vector.tensor_copy(warm_sink, pw[0:1, 0:1])
        # vaug memsets after the warm-up is queued (DVE-heavy)
        nc.vector.memset(vaug_sb, 0.0)
        nc.vector.memset(vaug_sb[:, :, :, :, 64:65], 1.0)

        # ---- projection half-unit slices (~1 us of PE each) ----
        def emit_q_slice(j, h):
            tq = j * TU + h * 256
            pq = aux_ps.tile([128, 256], f32, tag="aux", name=f"pq{j}_{h}")
            for c in range(NC):
                nc.tensor.matmul(pq, wq_sb[:, c, :],
                                 xall[:, j, c, h * 256:(h + 1) * 256],
                                 start=(c == 0), stop=(c == NC - 1))
            nc.vector.tensor_scalar_add(qt_sb[:, tq:tq + 256], pq, bq_sb)

        def emit_k_slice(j, h):
            tq = j * TU + h * 256
            pk = aux_ps.tile([128, 256], f32, tag="aux", name=f"pk{j}_{h}")
            for c in range(NC):
                nc.tensor.matmul(pk, wk_sb[:, c, :],
                                 xall[:, j, c, h * 256:(h + 1) * 256],
                                 start=(c == 0), stop=(c == NC - 1))
            nc.vector.tensor_copy(kt_sb[:, tq:tq + 256], pk)

        def emit_v_slice(j, h):
            pv = aux_ps.tile([128, 2, 128], f32, tag="aux",
                             name=f"pv{j}_{h}")
            for t in range(2):
                for c in range(NC):
                    nc.tensor.matmul(
                        pv[:, t, :],
                        xall[:, j, c,
                             h * 256 + t * 128:h * 256 + (t + 1) * 128],
                        wv_sb[:, c, :],
                        start=(c == 0), stop=(c == NC - 1))
            b = j // (NU // B)
            kt0 = (j % (NU // B)) * 4 + h * 2
            for t in range(2):
                nc.vector.tensor_copy(
                    vaug_sb[:, b, kt0 + t, 0, 0:64], pv[:, t, 0:64])
                nc.vector.tensor_copy(
                    vaug_sb[:, b, kt0 + t, 1, 0:64], pv[:, t, 64:128])

        SLICE_FN = {"q": emit_q_slice, "k": emit_k_slice, "v": emit_v_slice}

        # ---- attention steps: one per (b, qb, kt) ----
        steps = []
        for b in range(B):
            for qb in range(NQB):
                for kt in range(NKT):
                    steps.append((b, qb, kt))
        NSTEP = len(steps)

        score_ps = {}
        exp_sb = {}
        ctx_ps = {}
        ctx_sb = {}
        rs_sb = {}
        norm_rcp = {}

        def emit_scores(i):
            b, qb, kt = steps[i]
            pss = sc_ps.tile([128, 2, 512], f32, tag="sc", name=f"sc{i}")
            for h in range(2):
                nc.tensor.matmul(
                    pss[:, h, :],
                    kt_sb[h * 64:(h + 1) * 64,
                          b * S + kt * 128:b * S + (kt + 1) * 128],
                    qt_sb[h * 64:(h + 1) * 64,
                          b * S + qb * 512:b * S + (qb + 1) * 512],
                    start=True, stop=True)
            score_ps[i] = pss

        def emit_exp(i):
            et = etp.tile([128, 2, 512], bf16, tag="et", name=f"et{i}")
            nc.scalar.activation(et, score_ps.pop(i), EXP)
            exp_sb[i] = et

        workq = deque()   # dynamic drain items: norm halves / outproj

        def emit_ctx(i):
            b, qb, kt = steps[i]
            if kt == 0:
                ctx_ps[(b, qb, 0)] = pc_ps.tile(
                    [128, 512], f32, tag="pc", name=f"pc{i}h0")
                ctx_ps[(b, qb, 1)] = pc_ps.tile(
                    [128, 512], f32, tag="pc", name=f"pc{i}h1")
            et = exp_sb.pop(i)
            for h in range(2):
                nc.tensor.matmul(
                    ctx_ps[(b, qb, h)],
                    vaug_sb[:, b, kt, h, :],
                    et[:, h, :],
                    start=(kt == 0), stop=(kt == NKT - 1))
            if kt == NKT - 1:
                # drain ctx psum to SBUF immediately: frees both pc bufs
                # before the next q-block's first ctx matmul needs them;
                # rowsum rows go straight to f32 for the reciprocal
                cs = csb.tile([64, 2, 512], bf16, tag="cs",
                              name=f"cs{b}{qb}")
                rs = small.tile([1, 2, 512], f32, tag="rs",
                                name=f"rs{b}{qb}")
                pc0 = ctx_ps.pop((b, qb, 0))
                pc1 = ctx_ps.pop((b, qb, 1))
                nc.vector.tensor_copy(cs[:, 0, :], pc0[0:64, :])
                nc.vector.tensor_copy(rs[0:1, 0, :], pc0[64:65, :])
                nc.vector.tensor_copy(cs[:, 1, :], pc1[0:64, :])
                nc.vector.tensor_copy(rs[0:1, 1, :], pc1[64:65, :])
                ctx_sb[(b, qb)] = cs
                rs_sb[(b, qb)] = rs
                workq.appendleft(("norm_dve", b, qb))

        def emit_norm_dve(b, qb):
            """DVE half: reciprocal of the rowsums + bf16 cast."""
            rs = rs_sb.pop((b, qb))
            rcp = small.tile([1, 2, 512], f32, tag="rcp",
                             name=f"rcp{b}{qb}")
            nc.vector.reciprocal_approx_fast(rcp, rs)
            rcpb = small.tile([1, 2, 512], bf16, tag="rcpb",
                              name=f"rcpb{b}{qb}")
            nc.vector.tensor_copy(rcpb, rcp)
            norm_rcp[(b, qb)] = rcpb
            workq.append(("norm_pe", b, qb))

        def emit_norm_pe(b, qb):
            """PE half: broadcast matmuls + normalize into ctx2t."""
            cs = ctx_sb.pop((b, qb))
            rcpb = norm_rcp.pop((b, qb))
            dst = ctx2t_sb[:, b, qb * 512:(qb + 1) * 512]
            for h in range(2):
                pb = aux_ps.tile([64, 512], f32, tag="aux",
                                 name=f"pb{b}{qb}{h}")
                nc.tensor.matmul(pb, ones_bf, rcpb[0:1, h, :],
                                 start=True, stop=True)
                nc.vector.tensor_mul(dst[h * 64:(h + 1) * 64, :],
                                     cs[0:64, h, :], pb)
            for tt in range(4):
                workq.append(("op", b, qb, tt))

        def emit_outproj(b, qb, tt):
            tok0 = qb * 512 + tt * 128
            ysb = ypool.tile([128, 1024], bf16, tag="ysb",
                             name=f"ysb{b}{qb}{tt}")
            # split the psum->sbuf copies across DVE and the scalar
            # engine: halves the DVE queue depth so the qb-boundary
            # drain/reciprocal chains (which gate the ctx psum pool)
            # complete sooner; the last q-block rides scalar entirely
            last = (b == B - 1 and qb == NQB - 1)
            for wh in range(2):
                py = aux_ps.tile([128, 512], f32, tag="aux",
                                 name=f"py{b}{qb}{tt}{wh}")
                nc.tensor.matmul(
                    py, ctx2t_sb[:, b, tok0:tok0 + 128],
                    wo_sb[:, wh * 512:(wh + 1) * 512],
                    start=True, stop=True)
                if last or wh == 1:
                    nc.scalar.copy(ysb[:, wh * 512:(wh + 1) * 512], py)
                else:
                    nc.vector.tensor_copy(
                        ysb[:, wh * 512:(wh + 1) * 512], py)
            nc.sync.dma_start(
                yp_d[b * S + tok0:b * S + tok0 + 128, :], ysb)

        # ---- precomputed background slice plan: <=2/step, deadlines ----
        items = []
        for u in range(1, NU):
            base = 0 if u < NU // B else 64
            uu = u % (NU // B)
            for h in range(2):
                kt0 = uu * 4 + h * 2
                items.append((base + kt0 - 2, "k", u, h))
                items.append((base + kt0 + CTX_LAG - 1, "v", u, h))
                items.append((base + 16 * uu - 4 + h, "q", u, h))
        items.sort(key=lambda t: t[0])
        # earliest step unit j's x DMA has certainly landed
        ready = {j: max(2, 2 * (j - 3) + 2) for j in range(NU)}
        # one slice per step max: earliest-fit in deadline order keeps all
        # deadlines feasible and never doubles up a step's PE load
        sched = {}
        load = {}
        pace = 2.0
        for n, (dl, kind, j, h) in enumerate(items):
            # b0 items (dl<40) earliest-fit at their tight deadlines;
            # b1 items pace out so no step carries 2 slices
            s = max(ready[j], 2 if dl < 40 else int(16 + (n - 17) * pace))
            while load.get(s, 0) >= 1:
                s += 1
            sched.setdefault(s, []).append((kind, j, h))
            load[s] = load.get(s, 0) + 1

        def drain_work(i, nslices):
            while workq and workq[0][0] == "norm_dve":
                item = workq.popleft()
                emit_norm_dve(item[1], item[2])
            n = 0
            if nslices == 0:
                n = 2 if len(workq) >= 6 else 1
            elif nslices == 1:
                n = 1 if len(workq) >= 3 else 0
            else:
                n = 1 if len(workq) >= 7 else 0
            if i >= 96:
                n = max(n, 2)
            for _ in range(n):
                if not workq:
                    break
                item = workq.popleft()
                if item[0] == "norm_dve":
                    emit_norm_dve(item[1], item[2])
                elif item[0] == "norm_pe":
                    emit_norm_pe(item[1], item[2])
                else:
                    emit_outproj(item[1], item[2], item[3])

        # ---- prologue: minimal path to the first exp ----
        emit_k_slice(0, 0)
        emit_q_slice(0, 0)
        emit_q_slice(0, 1)
        emit_scores(0)
        emit_scores(1)
        emit_exp(0)
        emit_k_slice(0, 1)
        emit_v_slice(0, 0)
        emit_v_slice(0, 1)

        # ---- main loop: scores go LAST so their exp-semaphore wait is
        # pre-satisfied and the pipeline fill overlaps prior matmuls ----
        for i in range(2, NSTEP):
            emit_exp(i - 1)
            if i >= CTX_LAG:
                emit_ctx(i - CTX_LAG)
            sl = sched.get(i, [])
            for kind, j, h in sl:
                SLICE_FN[kind](j, h)
            drain_work(i, len(sl))
            emit_scores(i)
        emit_exp(NSTEP - 1)
        for i in range(NSTEP - CTX_LAG, NSTEP):
            emit_ctx(i)
        while workq:
            item = workq.popleft()
            if item[0] == "norm_dve":
                emit_norm_dve(item[1], item[2])
            elif item[0] == "norm_pe":
                emit_norm_pe(item[1], item[2])
            else:
                emit_outproj(item[1], item[2], item[3])

    nc.compile()
    return nc


_NC = None


def _to_bf16(a):
    import ml_dtypes
    return np.asarray(a, dtype=np.float32).astype(ml_dtypes.bfloat16)


def make_in_maps(inputs):
    """Build the 8 per-core input maps from full-precision inputs."""
    x = np.asarray(inputs["x"], dtype=np.float32)
    sc = 1.0 / np.sqrt(np.float32(DK))
    xT = np.ascontiguousarray(x.reshape(T, F).T)
    xT16 = _to_bf16(xT)
    in_maps = []
    for c in range(N_CORES):
        sl = slice(c * CF, (c + 1) * CF)
        in_maps.append({
            "xT": xT16,
            "Wq": _to_bf16(np.asarray(inputs["Wq"])[:, sl] * sc),
            "Wk": _to_bf16(np.asarray(inputs["Wk"])[:, sl]),
            "Wv": _to_bf16(np.asarray(inputs["Wv"])[:, sl]),
            "bq": np.ascontiguousarray(
                (np.asarray(inputs["bq"])[sl] * sc)
                .astype(np.float32).reshape(CF, 1)),
            "Wo": _to_bf16(np.asarray(inputs["Wo"])[sl, :]),
        })
    return in_maps


def combine_outputs(results, inputs):
    """Sum per-core bf16 partials, add host-side bias terms."""
    y = np.zeros((T, F), dtype=np.float64)
    for c in range(N_CORES):
        y += np.asarray(results[c]["yp"], dtype=np.float64)
    bo = np.asarray(inputs["bo"], dtype=np.float64)
    bv = np.asarray(inputs["bv"], dtype=np.float64)
    Wo = np.asarray(inputs["Wo"], dtype=np.float64)
    y += bo + bv @ Wo
    return y.astype(np.float32).reshape(B, S, F)


def kernel(x, Wq, bq, Wk, bk, Wv, bv, Wo, bo):
    global _NC
    if _NC is None:
        _NC = build_program()
    inputs = {"x": x, "Wq": Wq, "bq": bq, "Wk": Wk, "bk": bk,
              "Wv": Wv, "bv": bv, "Wo": Wo, "bo": bo}
    in_maps = make_in_maps(inputs)
    res = run_bass_kernel_spmd(_NC, in_maps, list(range(N_CORES)))
    return combine_outputs(res.results, inputs)


# revision 15
# speedup vs baseline: 1.0388x; 1.0388x over previous
"""Multi-head attention TRN2 kernel: 8-core head-sharded tensor parallelism.

Full inputs in, full output out. Each core computes 2 of the 16 heads:
QKV projection (its column slice), flash-style attention, and a partial
out-projection against its row slice of Wo. Host sums the 8 partials and
adds (bv @ Wo + bo) once; the K bias is dropped entirely (it only adds a
per-query constant to the logits, which softmax cancels).

v4 (all bf16, PE-bound): the PE carries ~1.45 us/step of work vs the
scalar engine's ~1.15 us exp, so the schedule's job is to keep the PE
queue dense with zero bursts. Background projections are half-unit
slices (~1 us) placed on a precomputed per-step plan (<=2 per step,
deadline-feasible, front-loaded); norm and out-projection drain at a
bounded per-step rate. Score pairs run concurrently on disjoint PE row
groups; ctx stationaries are zero-padded to 128 columns for FWL.
Prologue: dummy matmuls on memset data ramp the PE pstate while the
first x DMA lands. Tail: the last q-block's outproj copies ride the
then-idle scalar engine instead of the DVE.
"""
import sys

sys.path.insert(0, "/opt/trn_rl_repo")

from collections import deque
from contextlib import ExitStack

import numpy as np

import concourse.bass as bass
import concourse.tile as tile
from concourse import bacc, mybir
from concourse.bass_utils import run_bass_kernel_spmd

f32 = mybir.dt.float32
bf16 = mybir.dt.bfloat16
EXP = mybir.ActivationFunctionType.Exp

N_CORES = 8
B, S, F = 2, 2048, 1024
H = 16                 # heads total
DK = F // H            # 64
HPC = H // N_CORES     # 2 heads per core
CF = HPC * DK          # 128 = per-core slice of features
T = B * S              # 4096 tokens
TU = 512               # tokens per projection unit
NU = T // TU           # 8 projection units
NKT = S // 128         # 16 key tiles per sequence
NQB = S // 512         # 4 q-blocks per sequence
NC = F // 128          # 8 contraction chunks
CTX_LAG = 4


def build_program():
    nc = bacc.Bacc("TRN2", target_bir_lowering=False, debug=False,
                   num_devices=N_CORES)

    xt_d = nc.dram_tensor("xT", [F, T], bf16, kind="ExternalInput").ap()
    wq_d = nc.dram_tensor("Wq", [F, CF], bf16, kind="ExternalInput").ap()
    wk_d = nc.dram_tensor("Wk", [F, CF], bf16, kind="ExternalInput").ap()
    wv_d = nc.dram_tensor("Wv", [F, CF], bf16, kind="ExternalInput").ap()
    bq_d = nc.dram_tensor("bq", [CF, 1], f32, kind="ExternalInput").ap()
    wo_d = nc.dram_tensor("Wo", [CF, F], bf16, kind="ExternalInput").ap()
    yp_d = nc.dram_tensor("yp", [T, F], bf16, kind="ExternalOutput").ap()

    with tile.TileContext(nc) as tc, ExitStack() as ctx:
        const = ctx.enter_context(tc.tile_pool(name="const", bufs=1))
        big = ctx.enter_context(tc.tile_pool(name="big", bufs=1))
        etp = ctx.enter_context(tc.tile_pool(name="etp", bufs=6))
        csb = ctx.enter_context(tc.tile_pool(name="csb", bufs=2))
        small = ctx.enter_context(tc.tile_pool(name="small", bufs=4))
        ypool = ctx.enter_context(tc.tile_pool(name="ypool", bufs=4))

        # ---- persistent activations (all bf16) ----
        qt_sb = big.tile([128, T], bf16)       # [2 heads x 64 d, tokens]
        kt_sb = big.tile([128, T], bf16)
        # vaug per (b,kt,h): 128 cols = [V_h (64) | ones (1) | zeros (63)]
        # -> full-128-column stationary triggers FWL; ctx psum rows 65-127
        # are zeros and ignored.
        vaug_sb = big.tile([128, B, NKT, 2, 128], bf16)
        ctx2t_sb = big.tile([128, B, S], bf16)  # [2 heads x 64 d, b, tokens]

        warm_sb = const.tile([128, 512], bf16)
        nc.vector.memset(warm_sb, 0.0)
        ones_bf = const.tile([1, 64], bf16)
        nc.vector.memset(ones_bf, 1.0)

        # ---- inputs: DMA order puts the step-0 critical path first ----
        xall = const.tile([128, NU, NC, TU], bf16)
        wq_sb = const.tile([128, NC, CF], bf16)
        wk_sb = const.tile([128, NC, CF], bf16)
        wv_sb = const.tile([128, NC, CF], bf16)
        bq_sb = const.tile([128, 1], f32)
        wo_sb = const.tile([128, F], bf16)

        def dma_x(j):
            nc.sync.dma_start(
                xall[:, j, :, :],
                xt_d[:, j * TU:(j + 1) * TU]
                .rearrange("(a p) t -> p a t", p=128))

        dmadummy = const.tile([1, 4, 8], bf16)
        dma_x(0)
        nc.sync.dma_start(wq_sb, wq_d.rearrange("(a p) n -> p a n", p=128))
        nc.sync.dma_start(wk_sb, wk_d.rearrange("(a p) n -> p a n", p=128))
        nc.sync.dma_start(bq_sb, bq_d)
        # ring fillers: the DMA descriptor ring is ~8 deep, so these make
        # every later descriptor wait for x0's completion -> unit 0 gets
        # the full fabric bandwidth during the cold start
        for dd in range(4):
            nc.sync.dma_start(dmadummy[:, dd, :], xt_d[0:1, 0:8])
        dma_x(1)
        nc.sync.dma_start(wv_sb, wv_d.rearrange("(a p) n -> p a n", p=128))
        dma_x(2)
        nc.sync.dma_start(wo_sb, wo_d)
        for j in range(3, NU):
            dma_x(j)

        # ---- PSUM pools: 4 + 2 + 2 banks ----
        sc_ps = ctx.enter_context(
            tc.tile_pool(name="sc_ps", bufs=2, space="PSUM"))   # 4 banks
        pc_ps = ctx.enter_context(
            tc.tile_pool(name="pc_ps", bufs=2, space="PSUM"))   # 2 banks
        aux_ps = ctx.enter_context(
            tc.tile_pool(name="aux_ps", bufs=2, space="PSUM"))  # 2 banks

        # ---- PE warm-up: ramp the pstate while the x DMA lands ----
        pw = aux_ps.tile([128, 512], f32, tag="aux", name="warm")
        for w in range(16):
            nc.tensor.matmul(pw, warm_sb[:, 0:128], warm_sb,
                             start=(w == 0), stop=(w == 15))
        warm_sink = small.tile([1, 1], f32, tag="rs", name="warmsink")
        nc.vector.tensor_copy(warm_sink, pw[0:1, 0:1])
        # vaug memsets after the warm-up is queued (DVE-heavy)
        nc.vector.memset(vaug_sb, 0.0)
        nc.vector.memset(vaug_sb[:, :, :, :, 64:65], 1.0)

        # ---- projection half-unit slices (~1 us of PE each) ----
        def emit_q_slice(j, h):
            tq = j * TU + h * 256
            pq = aux_ps.tile([128, 256], f32, tag="aux", name=f"pq{j}_{h}")
            for c in range(NC):
                nc.tensor.matmul(pq, wq_sb[:, c, :],
                                 xall[:, j, c, h * 256:(h + 1) * 256],
                                 start=(c == 0), stop=(c == NC - 1))
            nc.vector.tensor_scalar_add(qt_sb[:, tq:tq + 256], pq, bq_sb)

        def emit_k_slice(j, h):
            tq = j * TU + h * 256
            pk = aux_ps.tile([128, 256], f32, tag="aux", name=f"pk{j}_{h}")
            for c in range(NC):
                nc.tensor.matmul(pk, wk_sb[:, c, :],
                                 xall[:, j, c, h * 256:(h + 1) * 256],
                                 start=(c == 0), stop=(c == NC - 1))
            nc.vector.tensor_copy(kt_sb[:, tq:tq + 256], pk)

        def emit_v_slice(j, h):
            pv = aux_ps.tile([128, 2, 128], f32, tag="aux",
                             name=f"pv{j}_{h}")
            for t in range(2):
                for c in range(NC):
                    nc.tensor.matmul(
                        pv[:, t, :],
                        xall[:, j, c,
                             h * 256 + t * 128:h * 256 + (t + 1) * 128],
                        wv_sb[:, c, :],
                        start=(c == 0), stop=(c == NC - 1))
            b = j // (NU // B)
            kt0 = (j % (NU // B)) * 4 + h * 2
            for t in range(2):
                nc.vector.tensor_copy(
                    vaug_sb[:, b, kt0 + t, 0, 0:64], pv[:, t, 0:64])
                nc.vector.tensor_copy(
                    vaug_sb[:, b, kt0 + t, 1, 0:64], pv[:, t, 64:128])

        SLICE_FN = {"q": emit_q_slice, "k": emit_k_slice, "v": emit_v_slice}

        # ---- attention steps: one per (b, qb, kt) ----
        steps = []
        for b in range(B):
            for qb in range(NQB):
                for kt in range(NKT):
                    steps.append((b, qb, kt))
        NSTEP = len(steps)

        score_ps = {}
        exp_sb = {}
        ctx_ps = {}
        ctx_sb = {}
        rs_sb = {}
        norm_rcp = {}

        def emit_scores(i):
            b, qb, kt = steps[i]
            pss = sc_ps.tile([128, 2, 512], f32, tag="sc", name=f"sc{i}")
            for h in range(2):
                nc.tensor.matmul(
                    pss[:, h, :],
                    kt_sb[h * 64:(h + 1) * 64,
                          b * S + kt * 128:b * S + (kt + 1) * 128],
                    qt_sb[h * 64:(h + 1) * 64,
                          b * S + qb * 512:b * S + (qb + 1) * 512],
                    start=True, stop=True)
            score_ps[i] = pss

        def emit_exp(i):
            et = etp.tile([128, 2, 512], bf16, tag="et", name=f"et{i}")
            nc.scalar.activation(et, score_ps.pop(i), EXP)
            exp_sb[i] = et

        workq = deque()   # dynamic drain items: norm halves / outproj

        def emit_ctx(i):
            b, qb, kt = steps[i]
            if kt == 0:
                ctx_ps[(b, qb, 0)] = pc_ps.tile(
                    [128, 512], f32, tag="pc", name=f"pc{i}h0")
                ctx_ps[(b, qb, 1)] = pc_ps.tile(
                    [128, 512], f32, tag="pc", name=f"pc{i}h1")
            et = exp_sb.pop(i)
            for h in range(2):
                nc.tensor.matmul(
                    ctx_ps[(b, qb, h)],
                    vaug_sb[:, b, kt, h, :],
                    et[:, h, :],
                    start=(kt == 0), stop=(kt == NKT - 1))
            if kt == NKT - 1:
                # drain ctx psum to SBUF immediately: frees both pc bufs
                # before the next q-block's first ctx matmul needs them;
                # rowsum rows go straight to f32 for the reciprocal
                cs = csb.tile([64, 2, 512], bf16, tag="cs",
                              name=f"cs{b}{qb}")
                rs = small.tile([1, 2, 512], f32, tag="rs",
                                name=f"rs{b}{qb}")
                pc0 = ctx_ps.pop((b, qb, 0))
                pc1 = ctx_ps.pop((b, qb, 1))
                nc.vector.tensor_copy(cs[:, 0, :], pc0[0:64, :])
                nc.vector.tensor_copy(rs[0:1, 0, :], pc0[64:65, :])
                nc.vector.tensor_copy(cs[:, 1, :], pc1[0:64, :])
                nc.vector.tensor_copy(rs[0:1, 1, :], pc1[64:65, :])
                ctx_sb[(b, qb)] = cs
                rs_sb[(b, qb)] = rs
                workq.appendleft(("norm_dve", b, qb))

        def emit_norm_dve(b, qb):
            """DVE half: reciprocal of the rowsums + bf16 cast."""
            rs = rs_sb.pop((b, qb))
            rcp = small.tile([1, 2, 512], f32, tag="rcp",
                             name=f"rcp{b}{qb}")
            nc.vector.reciprocal_approx_fast(rcp, rs)
            rcpb = small.tile([1, 2, 512], bf16, tag="rcpb",
                              name=f"rcpb{b}{qb}")
            nc.vector.tensor_copy(rcpb, rcp)
            norm_rcp[(b, qb)] = rcpb
            workq.append(("norm_pe", b, qb))

        def emit_norm_pe(b, qb):
            """PE half: broadcast matmuls + normalize into ctx2t."""
            cs = ctx_sb.pop((b, qb))
            rcpb = norm_rcp.pop((b, qb))
            dst = ctx2t_sb[:, b, qb * 512:(qb + 1) * 512]
            for h in range(2):
                pb = aux_ps.tile([64, 512], f32, tag="aux",
                                 name=f"pb{b}{qb}{h}")
                nc.tensor.matmul(pb, ones_bf, rcpb[0:1, h, :],
                                 start=True, stop=True)
                nc.vector.tensor_mul(dst[h * 64:(h + 1) * 64, :],
                                     cs[0:64, h, :], pb)
            for tt in range(4):
                workq.append(("op", b, qb, tt))

        def emit_outproj(b, qb, tt):
            tok0 = qb * 512 + tt * 128
            ysb = ypool.tile([128, 1024], bf16, tag="ysb",
                             name=f"ysb{b}{qb}{tt}")
            # the last q-block's copies alternate between the then-idle
            # scalar engine and the DVE so the tail drains in parallel
            last = (b == B - 1 and qb == NQB - 1)
            for wh in range(2):
                py = aux_ps.tile([128, 512], f32, tag="aux",
                                 name=f"py{b}{qb}{tt}{wh}")
                nc.tensor.matmul(
                    py, ctx2t_sb[:, b, tok0:tok0 + 128],
                    wo_sb[:, wh * 512:(wh + 1) * 512],
                    start=True, stop=True)
                if last and wh == 1:
                    nc.scalar.copy(ysb[:, wh * 512:(wh + 1) * 512], py)
                else:
                    nc.vector.tensor_copy(
                        ysb[:, wh * 512:(wh + 1) * 512], py)
            nc.sync.dma_start(
                yp_d[b * S + tok0:b * S + tok0 + 128, :], ysb)

        # ---- precomputed background slice plan: <=2/step, deadlines ----
        items = []
        for u in range(1, NU):
            base = 0 if u < NU // B else 64
            uu = u % (NU // B)
            for h in range(2):
                kt0 = uu * 4 + h * 2
                items.append((base + kt0 - 2, "k", u, h))
                items.append((base + kt0 + CTX_LAG - 1, "v", u, h))
                items.append((base + 16 * uu - 4 + h, "q", u, h))
        items.sort(key=lambda t: t[0])
        # earliest step unit j's x DMA has certainly landed
        ready = {j: max(2, 2 * (j - 3) + 2) for j in range(NU)}
        # one slice per step max: earliest-fit in deadline order keeps all
        # deadlines feasible and never doubles up a step's PE load
        sched = {}
        load = {}
        nxt = 2
        for dl, kind, j, h in items:
            s = max(ready[j], nxt)
            while load.get(s, 0) >= 1:
                s += 1
            assert s <= max(dl, ready[j]) + 6, (s, dl, kind, j, h)
            sched.setdefault(s, []).append((kind, j, h))
            load[s] = load.get(s, 0) + 1

        def drain_work(i, nslices):
            while workq and workq[0][0] == "norm_dve":
                item = workq.popleft()
                emit_norm_dve(item[1], item[2])
            n = 0
            if nslices == 0:
                n = 2 if len(workq) >= 6 else 1
            elif nslices == 1:
                n = 1 if len(workq) >= 3 else 0
            else:
                n = 1 if len(workq) >= 7 else 0
            if i >= 96:
                n = max(n, 2)
            for _ in range(n):
                if not workq:
                    break
                item = workq.popleft()
                if item[0] == "norm_dve":
                    emit_norm_dve(item[1], item[2])
                elif item[0] == "norm_pe":
                    emit_norm_pe(item[1], item[2])
                else:
                    emit_outproj(item[1], item[2], item[3])

        # ---- prologue: minimal path to the first exp ----
        emit_k_slice(0, 0)
        emit_q_slice(0, 0)
        emit_q_slice(0, 1)
        emit_scores(0)
        emit_scores(1)
        emit_exp(0)
        emit_k_slice(0, 1)
        emit_v_slice(0, 0)
        emit_v_slice(0, 1)

        # ---- main loop: scores go LAST so their exp-semaphore wait is
        # pre-satisfied and the pipeline fill overlaps prior matmuls ----
        for i in range(2, NSTEP):
            emit_exp(i - 1)
            if i >= CTX_LAG:
                emit_ctx(i - CTX_LAG)
            sl = sched.get(i, [])
            for kind, j, h in sl:
                SLICE_FN[kind](j, h)
            drain_work(i, len(sl))
            emit_scores(i)
        emit_exp(NSTEP - 1)
        for i in range(NSTEP - CTX_LAG, NSTEP):
            emit_ctx(i)
        while workq:
            item = workq.popleft()
            if item[0] == "norm_dve":
                emit_norm_dve(item[1], item[2])
            elif item[0] == "norm_pe":
                emit_norm_pe(item[1], item[2])
            else:
                emit_outproj(item[1], item[2], item[3])

    nc.compile()
    return nc


_NC = None


def _to_bf16(a):
    import ml_dtypes
    return np.asarray(a, dtype=np.float32).astype(ml_dtypes.bfloat16)


def make_in_maps(inputs):
    """Build the 8 per-core input maps from full-precision inputs."""
    x = np.asarray(inputs["x"], dtype=np.float32)
    sc = 1.0 / np.sqrt(np.float32(DK))
    xT = np.ascontiguousarray(x.reshape(T, F).T)
    xT16 = _to_bf16(xT)
    in_maps = []
    for c in range(N_CORES):
        sl = slice(c * CF, (c + 1) * CF)
        in_maps.append({
            "xT": xT16,
            "Wq": _to_bf16(np.asarray(inputs["Wq"])[:, sl] * sc),
            "Wk": _to_bf16(np.asarray(inputs["Wk"])[:, sl]),
            "Wv": _to_bf16(np.asarray(inputs["Wv"])[:, sl]),
            "bq": np.ascontiguousarray(
                (np.asarray(inputs["bq"])[sl] * sc)
                .astype(np.float32).reshape(CF, 1)),
            "Wo": _to_bf16(np.asarray(inputs["Wo"])[sl, :]),
        })
    return in_maps


def combine_outputs(results, inputs):
    """Sum per-core bf16 partials, add host-side bias terms."""
    y = np.zeros((T, F), dtype=np.float64)
    for c in range(N_CORES):
        y += np.asarray(results[c]["yp"], dtype=np.float64)
    bo = np.asarray(inputs["bo"], dtype=np.float64)
    bv = np.asarray(inputs["bv"], dtype=np.float64)
    Wo = np.asarray(inputs["Wo"], dtype=np.float64)
    y += bo + bv @ Wo
    return y.astype(np.float32).reshape(B, S, F)


def kernel(x, Wq, bq, Wk, bk, Wv, bv, Wo, bo):
    global _NC
    if _NC is None:
        _NC = build_program()
    inputs = {"x": x, "Wq": Wq, "bq": bq, "Wk": Wk, "bk": bk,
              "Wv": Wv, "bv": bv, "Wo": Wo, "bo": bo}
    in_maps = make_in_maps(inputs)
    res = run_bass_kernel_spmd(_NC, in_maps, list(range(N_CORES)))
    return combine_outputs(res.results, inputs)


# revision 16
# speedup vs baseline: 1.0464x; 1.0073x over previous
"""Multi-head attention TRN2 kernel: 8-core head-sharded tensor parallelism.

Full inputs in, full output out. Each core computes 2 of the 16 heads:
QKV projection (its column slice), flash-style attention, and a partial
out-projection against its row slice of Wo. Host sums the 8 partials and
adds (bv @ Wo + bo) once; the K bias is dropped entirely (it only adds a
per-query constant to the logits, which softmax cancels).

v4 (all bf16, PE-bound): the PE carries ~1.45 us/step of work vs the
scalar engine's ~1.15 us exp, so the schedule's job is to keep the PE
queue dense with zero bursts. Background projections are half-unit
slices (~1 us) placed on a precomputed per-step plan (<=2 per step,
deadline-feasible, front-loaded); norm and out-projection drain at a
bounded per-step rate. Score pairs run concurrently on disjoint PE row
groups; ctx stationaries are zero-padded to 128 columns for FWL.
Prologue: dummy matmuls on memset data ramp the PE pstate while the
first x DMA lands. Tail: the last q-block's outproj copies ride the
then-idle scalar engine instead of the DVE.
"""
import sys

sys.path.insert(0, "/opt/trn_rl_repo")

from collections import deque
from contextlib import ExitStack

import numpy as np

import concourse.bass as bass
import concourse.tile as tile
from concourse import bacc, mybir
from concourse.bass_utils import run_bass_kernel_spmd

f32 = mybir.dt.float32
bf16 = mybir.dt.bfloat16
EXP = mybir.ActivationFunctionType.Exp

N_CORES = 8
B, S, F = 2, 2048, 1024
H = 16                 # heads total
DK = F // H            # 64
HPC = H // N_CORES     # 2 heads per core
CF = HPC * DK          # 128 = per-core slice of features
T = B * S              # 4096 tokens
TU = 512               # tokens per projection unit
NU = T // TU           # 8 projection units
NKT = S // 128         # 16 key tiles per sequence
NQB = S // 512         # 4 q-blocks per sequence
NC = F // 128          # 8 contraction chunks
CTX_LAG = 4


def build_program():
    nc = bacc.Bacc("TRN2", target_bir_lowering=False, debug=False,
                   num_devices=N_CORES)

    xt_d = nc.dram_tensor("xT", [F, T], bf16, kind="ExternalInput").ap()
    wq_d = nc.dram_tensor("Wq", [F, CF], bf16, kind="ExternalInput").ap()
    wk_d = nc.dram_tensor("Wk", [F, CF], bf16, kind="ExternalInput").ap()
    wv_d = nc.dram_tensor("Wv", [F, CF], bf16, kind="ExternalInput").ap()
    bq_d = nc.dram_tensor("bq", [CF, 1], f32, kind="ExternalInput").ap()
    wo_d = nc.dram_tensor("Wo", [CF, F], bf16, kind="ExternalInput").ap()
    yp_d = nc.dram_tensor("yp", [T, F], bf16, kind="ExternalOutput").ap()

    with tile.TileContext(nc) as tc, ExitStack() as ctx:
        const = ctx.enter_context(tc.tile_pool(name="const", bufs=1))
        big = ctx.enter_context(tc.tile_pool(name="big", bufs=1))
        etp = ctx.enter_context(tc.tile_pool(name="etp", bufs=6))
        csb = ctx.enter_context(tc.tile_pool(name="csb", bufs=2))
        small = ctx.enter_context(tc.tile_pool(name="small", bufs=4))
        ypool = ctx.enter_context(tc.tile_pool(name="ypool", bufs=4))

        # ---- persistent activations (all bf16) ----
        qt_sb = big.tile([128, T], bf16)       # [2 heads x 64 d, tokens]
        kt_sb = big.tile([128, T], bf16)
        # vaug per (b,kt,h): 128 cols = [V_h (64) | ones (1) | zeros (63)]
        # -> full-128-column stationary triggers FWL; ctx psum rows 65-127
        # are zeros and ignored.
        vaug_sb = big.tile([128, B, NKT, 2, 128], bf16)
        ctx2t_sb = big.tile([128, B, S], bf16)  # [2 heads x 64 d, b, tokens]

        warm_sb = const.tile([128, 512], bf16)
        nc.vector.memset(warm_sb, 0.0)
        ones_bf = const.tile([1, 64], bf16)
        nc.vector.memset(ones_bf, 1.0)

        # ---- inputs: DMA order puts the step-0 critical path first ----
        xall = const.tile([128, NU, NC, TU], bf16)
        wq_sb = const.tile([128, NC, CF], bf16)
        wk_sb = const.tile([128, NC, CF], bf16)
        wv_sb = const.tile([128, NC, CF], bf16)
        bq_sb = const.tile([128, 1], f32)
        wo_sb = const.tile([128, F], bf16)

        def dma_x(j):
            nc.sync.dma_start(
                xall[:, j, :, :],
                xt_d[:, j * TU:(j + 1) * TU]
                .rearrange("(a p) t -> p a t", p=128))

        dmadummy = const.tile([1, 4, 8], bf16)
        # unit 0 in two token-halves: k0/q0 slice h=0 starts ~3us earlier
        for hh in range(2):
            nc.sync.dma_start(
                xall[:, 0, :, hh * 256:(hh + 1) * 256],
                xt_d[:, hh * 256:(hh + 1) * 256]
                .rearrange("(a p) t -> p a t", p=128))
        nc.sync.dma_start(wq_sb, wq_d.rearrange("(a p) n -> p a n", p=128))
        nc.sync.dma_start(wk_sb, wk_d.rearrange("(a p) n -> p a n", p=128))
        nc.sync.dma_start(bq_sb, bq_d)
        # ring fillers: the DMA descriptor ring is ~8 deep, so these make
        # every later descriptor wait for x0's completion -> unit 0 gets
        # the full fabric bandwidth during the cold start
        for dd in range(4):
            nc.sync.dma_start(dmadummy[:, dd, :], xt_d[0:1, 0:8])
        dma_x(1)
        nc.sync.dma_start(wv_sb, wv_d.rearrange("(a p) n -> p a n", p=128))
        dma_x(2)
        nc.sync.dma_start(wo_sb, wo_d)
        for j in range(3, NU):
            dma_x(j)

        # ---- PSUM pools: 4 + 2 + 2 banks ----
        sc_ps = ctx.enter_context(
            tc.tile_pool(name="sc_ps", bufs=2, space="PSUM"))   # 4 banks
        pc_ps = ctx.enter_context(
            tc.tile_pool(name="pc_ps", bufs=2, space="PSUM"))   # 2 banks
        aux_ps = ctx.enter_context(
            tc.tile_pool(name="aux_ps", bufs=2, space="PSUM"))  # 2 banks

        # ---- PE warm-up: ramp the pstate while the x DMA lands ----
        pw = aux_ps.tile([128, 512], f32, tag="aux", name="warm")
        for w in range(16):
            nc.tensor.matmul(pw, warm_sb[:, 0:128], warm_sb,
                             start=(w == 0), stop=(w == 15))
        warm_sink = small.tile([1, 1], f32, tag="rs", name="warmsink")
        nc.vector.tensor_copy(warm_sink, pw[0:1, 0:1])
        # vaug memsets after the warm-up is queued (DVE-heavy)
        nc.vector.memset(vaug_sb, 0.0)
        nc.vector.memset(vaug_sb[:, :, :, :, 64:65], 1.0)

        # ---- projection half-unit slices (~1 us of PE each) ----
        def emit_q_slice(j, h):
            tq = j * TU + h * 256
            pq = aux_ps.tile([128, 256], f32, tag="aux", name=f"pq{j}_{h}")
            for c in range(NC):
                nc.tensor.matmul(pq, wq_sb[:, c, :],
                                 xall[:, j, c, h * 256:(h + 1) * 256],
                                 start=(c == 0), stop=(c == NC - 1))
            nc.vector.tensor_scalar_add(qt_sb[:, tq:tq + 256], pq, bq_sb)

        def emit_k_slice(j, h):
            tq = j * TU + h * 256
            pk = aux_ps.tile([128, 256], f32, tag="aux", name=f"pk{j}_{h}")
            for c in range(NC):
                nc.tensor.matmul(pk, wk_sb[:, c, :],
                                 xall[:, j, c, h * 256:(h + 1) * 256],
                                 start=(c == 0), stop=(c == NC - 1))
            nc.vector.tensor_copy(kt_sb[:, tq:tq + 256], pk)

        def emit_v_slice(j, h):
            pv = aux_ps.tile([128, 2, 128], f32, tag="aux",
                             name=f"pv{j}_{h}")
            for t in range(2):
                for c in range(NC):
                    nc.tensor.matmul(
                        pv[:, t, :],
                        xall[:, j, c,
                             h * 256 + t * 128:h * 256 + (t + 1) * 128],
                        wv_sb[:, c, :],
                        start=(c == 0), stop=(c == NC - 1))
            b = j // (NU // B)
            kt0 = (j % (NU // B)) * 4 + h * 2
            for t in range(2):
                nc.vector.tensor_copy(
                    vaug_sb[:, b, kt0 + t, 0, 0:64], pv[:, t, 0:64])
                nc.vector.tensor_copy(
                    vaug_sb[:, b, kt0 + t, 1, 0:64], pv[:, t, 64:128])

        SLICE_FN = {"q": emit_q_slice, "k": emit_k_slice, "v": emit_v_slice}

        # ---- attention steps: one per (b, qb, kt) ----
        steps = []
        for b in range(B):
            for qb in range(NQB):
                for kt in range(NKT):
                    steps.append((b, qb, kt))
        NSTEP = len(steps)

        score_ps = {}
        exp_sb = {}
        ctx_ps = {}
        ctx_sb = {}
        rs_sb = {}
        norm_rcp = {}

        def emit_scores(i):
            b, qb, kt = steps[i]
            pss = sc_ps.tile([128, 2, 512], f32, tag="sc", name=f"sc{i}")
            for h in range(2):
                nc.tensor.matmul(
                    pss[:, h, :],
                    kt_sb[h * 64:(h + 1) * 64,
                          b * S + kt * 128:b * S + (kt + 1) * 128],
                    qt_sb[h * 64:(h + 1) * 64,
                          b * S + qb * 512:b * S + (qb + 1) * 512],
                    start=True, stop=True)
            score_ps[i] = pss

        def emit_exp(i):
            et = etp.tile([128, 2, 512], bf16, tag="et", name=f"et{i}")
            nc.scalar.activation(et, score_ps.pop(i), EXP)
            exp_sb[i] = et

        workq = deque()   # dynamic drain items: norm halves / outproj

        def emit_ctx(i):
            b, qb, kt = steps[i]
            if kt == 0:
                ctx_ps[(b, qb, 0)] = pc_ps.tile(
                    [128, 512], f32, tag="pc", name=f"pc{i}h0")
                ctx_ps[(b, qb, 1)] = pc_ps.tile(
                    [128, 512], f32, tag="pc", name=f"pc{i}h1")
            et = exp_sb.pop(i)
            for h in range(2):
                nc.tensor.matmul(
                    ctx_ps[(b, qb, h)],
                    vaug_sb[:, b, kt, h, :],
                    et[:, h, :],
                    start=(kt == 0), stop=(kt == NKT - 1))
            if kt == NKT - 1:
                # drain ctx psum to SBUF immediately: frees both pc bufs
                # before the next q-block's first ctx matmul needs them;
                # rowsum rows go straight to f32 for the reciprocal
                cs = csb.tile([64, 2, 512], bf16, tag="cs",
                              name=f"cs{b}{qb}")
                rs = small.tile([1, 2, 512], f32, tag="rs",
                                name=f"rs{b}{qb}")
                pc0 = ctx_ps.pop((b, qb, 0))
                pc1 = ctx_ps.pop((b, qb, 1))
                nc.vector.tensor_copy(cs[:, 0, :], pc0[0:64, :])
                nc.vector.tensor_copy(rs[0:1, 0, :], pc0[64:65, :])
                nc.vector.tensor_copy(cs[:, 1, :], pc1[0:64, :])
                nc.vector.tensor_copy(rs[0:1, 1, :], pc1[64:65, :])
                ctx_sb[(b, qb)] = cs
                rs_sb[(b, qb)] = rs
                workq.appendleft(("norm_dve", b, qb))

        def emit_norm_dve(b, qb):
            """DVE half: reciprocal of the rowsums + bf16 cast."""
            rs = rs_sb.pop((b, qb))
            rcp = small.tile([1, 2, 512], f32, tag="rcp",
                             name=f"rcp{b}{qb}")
            nc.vector.reciprocal_approx_fast(rcp, rs)
            rcpb = small.tile([1, 2, 512], bf16, tag="rcpb",
                              name=f"rcpb{b}{qb}")
            nc.vector.tensor_copy(rcpb, rcp)
            norm_rcp[(b, qb)] = rcpb
            workq.append(("norm_pe", b, qb))

        def emit_norm_pe(b, qb):
            """PE half: broadcast matmuls + normalize into ctx2t."""
            cs = ctx_sb.pop((b, qb))
            rcpb = norm_rcp.pop((b, qb))
            dst = ctx2t_sb[:, b, qb * 512:(qb + 1) * 512]
            for h in range(2):
                pb = aux_ps.tile([64, 512], f32, tag="aux",
                                 name=f"pb{b}{qb}{h}")
                nc.tensor.matmul(pb, ones_bf, rcpb[0:1, h, :],
                                 start=True, stop=True)
                nc.vector.tensor_mul(dst[h * 64:(h + 1) * 64, :],
                                     cs[0:64, h, :], pb)
            for tt in range(4):
                workq.append(("op", b, qb, tt))

        def emit_outproj(b, qb, tt):
            tok0 = qb * 512 + tt * 128
            ysb = ypool.tile([128, 1024], bf16, tag="ysb",
                             name=f"ysb{b}{qb}{tt}")
            # the last q-block's copies alternate between the then-idle
            # scalar engine and the DVE so the tail drains in parallel
            last = (b == B - 1 and qb == NQB - 1)
            for wh in range(2):
                py = aux_ps.tile([128, 512], f32, tag="aux",
                                 name=f"py{b}{qb}{tt}{wh}")
                nc.tensor.matmul(
                    py, ctx2t_sb[:, b, tok0:tok0 + 128],
                    wo_sb[:, wh * 512:(wh + 1) * 512],
                    start=True, stop=True)
                if last and wh == 1:
                    nc.scalar.copy(ysb[:, wh * 512:(wh + 1) * 512], py)
                else:
                    nc.vector.tensor_copy(
                        ysb[:, wh * 512:(wh + 1) * 512], py)
            nc.sync.dma_start(
                yp_d[b * S + tok0:b * S + tok0 + 128, :], ysb)

        # ---- precomputed background slice plan: <=2/step, deadlines ----
        items = []
        for u in range(1, NU):
            base = 0 if u < NU // B else 64
            uu = u % (NU // B)
            for h in range(2):
                kt0 = uu * 4 + h * 2
                items.append((base + kt0 - 2, "k", u, h))
                items.append((base + kt0 + CTX_LAG - 1, "v", u, h))
                items.append((base + 16 * uu - 4 + h, "q", u, h))
        items.sort(key=lambda t: t[0])
        # earliest step unit j's x DMA has certainly landed
        ready = {j: max(2, 2 * (j - 3) + 2) for j in range(NU)}
        # one slice per step max: earliest-fit in deadline order keeps all
        # deadlines feasible and never doubles up a step's PE load
        sched = {}
        load = {}
        nxt = 2
        for dl, kind, j, h in items:
            s = max(ready[j], nxt)
            while load.get(s, 0) >= 1:
                s += 1
            assert s <= max(dl, ready[j]) + 6, (s, dl, kind, j, h)
            sched.setdefault(s, []).append((kind, j, h))
            load[s] = load.get(s, 0) + 1

        def drain_work(i, nslices):
            while workq and workq[0][0] == "norm_dve":
                item = workq.popleft()
                emit_norm_dve(item[1], item[2])
            n = 0
            if nslices == 0:
                n = 2 if len(workq) >= 6 else 1
            elif nslices == 1:
                n = 1 if len(workq) >= 3 else 0
            else:
                n = 1 if len(workq) >= 7 else 0
            if i >= 96:
                n = max(n, 2)
            for _ in range(n):
                if not workq:
                    break
                item = workq.popleft()
                if item[0] == "norm_dve":
                    emit_norm_dve(item[1], item[2])
                elif item[0] == "norm_pe":
                    emit_norm_pe(item[1], item[2])
                else:
                    emit_outproj(item[1], item[2], item[3])

        # ---- prologue: minimal path to the first exp ----
        emit_k_slice(0, 0)
        emit_q_slice(0, 0)
        emit_q_slice(0, 1)
        emit_scores(0)
        emit_scores(1)
        emit_exp(0)
        emit_k_slice(0, 1)
        emit_v_slice(0, 0)
        emit_v_slice(0, 1)

        # graduated ctx lag: 4 normally, 3 for kt 12-13, 2 for kt 14-15,
        # so the end-of-qblock psum drains start two steps earlier
        next_ctx = [0]

        def ctx_lag_for(j):
            kt = steps[j][2]
            return 4 if kt <= 11 else (3 if kt <= 13 else 2)

        def drain_ctx(i):
            while (next_ctx[0] < NSTEP
                   and next_ctx[0] + ctx_lag_for(next_ctx[0]) <= i):
                emit_ctx(next_ctx[0])
                next_ctx[0] += 1

        # ---- main loop: scores go LAST so their exp-semaphore wait is
        # pre-satisfied and the pipeline fill overlaps prior matmuls ----
        for i in range(2, NSTEP):
            emit_exp(i - 1)
            drain_ctx(i)
            sl = sched.get(i, [])
            for kind, j, h in sl:
                SLICE_FN[kind](j, h)
            drain_work(i, len(sl))
            emit_scores(i)
        emit_exp(NSTEP - 1)
        while next_ctx[0] < NSTEP:
            emit_ctx(next_ctx[0])
            next_ctx[0] += 1
        while workq:
            item = workq.popleft()
            if item[0] == "norm_dve":
                emit_norm_dve(item[1], item[2])
            elif item[0] == "norm_pe":
                emit_norm_pe(item[1], item[2])
            else:
                emit_outproj(item[1], item[2], item[3])

    nc.compile()
    return nc


_NC = None


def _to_bf16(a):
    import ml_dtypes
    return np.asarray(a, dtype=np.float32).astype(ml_dtypes.bfloat16)


def make_in_maps(inputs):
    """Build the 8 per-core input maps from full-precision inputs."""
    x = np.asarray(inputs["x"], dtype=np.float32)
    sc = 1.0 / np.sqrt(np.float32(DK))
    xT = np.ascontiguousarray(x.reshape(T, F).T)
    xT16 = _to_bf16(xT)
    in_maps = []
    for c in range(N_CORES):
        sl = slice(c * CF, (c + 1) * CF)
        in_maps.append({
            "xT": xT16,
            "Wq": _to_bf16(np.asarray(inputs["Wq"])[:, sl] * sc),
            "Wk": _to_bf16(np.asarray(inputs["Wk"])[:, sl]),
            "Wv": _to_bf16(np.asarray(inputs["Wv"])[:, sl]),
            "bq": np.ascontiguousarray(
                (np.asarray(inputs["bq"])[sl] * sc)
                .astype(np.float32).reshape(CF, 1)),
            "Wo": _to_bf16(np.asarray(inputs["Wo"])[sl, :]),
        })
    return in_maps


def combine_outputs(results, inputs):
    """Sum per-core bf16 partials, add host-side bias terms."""
    y = np.zeros((T, F), dtype=np.float64)
    for c in range(N_CORES):
        y += np.asarray(results[c]["yp"], dtype=np.float64)
    bo = np.asarray(inputs["bo"], dtype=np.float64)
    bv = np.asarray(inputs["bv"], dtype=np.float64)
    Wo = np.asarray(inputs["Wo"], dtype=np.float64)
    y += bo + bv @ Wo
    return y.astype(np.float32).reshape(B, S, F)


def kernel(x, Wq, bq, Wk, bk, Wv, bv, Wo, bo):
    global _NC
    if _NC is None:
        _NC = build_program()
    inputs = {"x": x, "Wq": Wq, "bq": bq, "Wk": Wk, "bk": bk,
              "Wv": Wv, "bv": bv, "Wo": Wo, "bo": bo}
    in_maps = make_in_maps(inputs)
    res = run_bass_kernel_spmd(_NC, in_maps, list(range(N_CORES)))
    return combine_outputs(res.results, inputs)
